# revision 9
# baseline (speedup 1.0000x reference)
"""CrossModalAttention Trainium2 kernel.

Data-parallel over batch B=8 across the 8 NeuronCores (core b owns batch b,
weights replicated, no collectives). Within a core all 9 modality-pair
attentions run with bf16 matmuls / fp32 PSUM accumulation.

Layout strategy (per core, batch b fixed):
  xT[m]  : [c, n]  (c on partitions)  -- host pre-transposed, bf16
  qT/kT[m]: [d, n] = Wq[m].T-projection output, d on partitions
  v[m]   : [n_k, d] natural layout, with an extra per-head "ones" column so
           the PV matmul produces the softmax denominator Z in column 64.
  S^T    : [k, n] per (i,j,head) from lhsT=kT-slice, rhs=qT-slice (K=64)
  E = exp(S^T) via ACT, bf16
  PV     : out[n-sub, 65] with lhsT=E-slice, rhs=v-slice(65 cols) accumulated
           over key tiles; col 64 = Z * ones_val.
  consume: fused[n, c] += PV[:, 0:64] * reciprocal(Z*ones_val)  (DVE)
  final  : PE-transpose fused -> fusedT, out = fusedT.T @ Wp + bp
"""

import sys

import numpy as np

for _p in ("/opt/trn_rl_repo",):
    if _p not in sys.path:
        sys.path.insert(0, _p)

import ml_dtypes  # noqa: E402

import concourse.bass as bass  # noqa: E402
from concourse import bacc  # noqa: E402
import concourse.mybir as mybir  # noqa: E402
import concourse.tile as tile  # noqa: E402

M, B, N, C, H = 3, 8, 512, 512, 8
HD = C // H  # 64
P = 128
CT = C // P  # 4 contraction tiles
NT = N // P  # 4 row tiles
DT = C // P  # 4 output-channel tiles
SCALE = float(HD) ** -0.5

BF16 = mybir.dt.bfloat16
F32 = mybir.dt.float32
NP_BF16 = ml_dtypes.bfloat16

AluOp = mybir.AluOpType
ActFn = mybir.ActivationFunctionType


def _build_bass(mw, uniform, ones_val, reps=1):
    """Emit the single-core SPMD program. mw is the [M,M] modal weight matrix
    (values are baked into the program as immediates)."""
    from concourse.masks import make_identity

    nc = bacc.Bacc(None)

    xt_d = nc.dram_tensor("xt", [M, P, CT, N], BF16, kind="ExternalInput")
    wq_d = nc.dram_tensor("wq", [M, P, CT, C], BF16, kind="ExternalInput")
    wk_d = nc.dram_tensor("wk", [M, P, CT, C], BF16, kind="ExternalInput")
    wv_d = nc.dram_tensor("wv", [M, P, CT, C], BF16, kind="ExternalInput")
    wp_d = nc.dram_tensor("wp", [P, CT, C], BF16, kind="ExternalInput")
    bq_d = nc.dram_tensor("bq", [M, P, DT], F32, kind="ExternalInput")
    bk_d = nc.dram_tensor("bk", [M, P, DT], F32, kind="ExternalInput")
    bv_d = nc.dram_tensor("bv", [M, 1, C], BF16, kind="ExternalInput")
    bp_d = nc.dram_tensor("bp", [1, C], BF16, kind="ExternalInput")
    out_d = nc.dram_tensor("out", [N, C], F32, kind="ExternalOutput")

    with tile.TileContext(nc) as tc:
        with (
            tc.tile_pool(name="consts", bufs=1) as consts,
            tc.tile_pool(name="esb", bufs=3) as esb,
            tc.tile_pool(name="zr", bufs=8) as zrp,
            tc.tile_pool(name="ppsum", bufs=2, space="PSUM") as ppsum,
            tc.tile_pool(name="spsum", bufs=2, space="PSUM") as spsum,
            tc.tile_pool(name="pvpsum", bufs=1, space="PSUM") as pvpsum,
        ):
            # ---- persistent SBUF tiles ----
            xt_sb = [consts.tile([P, CT, N], BF16, tag=f"xt{m}", name=f"xt{m}") for m in range(M)]
            wq_sb = [consts.tile([P, CT, C], BF16, tag=f"wq{m}", name=f"wq{m}") for m in range(M)]
            wk_sb = [consts.tile([P, CT, C], BF16, tag=f"wk{m}", name=f"wk{m}") for m in range(M)]
            wv_sb = [consts.tile([P, CT, C], BF16, tag=f"wv{m}", name=f"wv{m}") for m in range(M)]
            wp_sb = consts.tile([P, CT, C], BF16, tag="wp", name="wp")
            bq_sb = [consts.tile([P, DT], F32, tag=f"bq{m}", name=f"bq{m}") for m in range(M)]
            bk_sb = [consts.tile([P, DT], F32, tag=f"bk{m}", name=f"bk{m}") for m in range(M)]
            bv_sb = [consts.tile([1, C], BF16, tag=f"bv{m}", name=f"bv{m}") for m in range(M)]
            bp_sb = consts.tile([1, C], BF16, tag="bp", name="bp")
            qT_sb = [consts.tile([P, DT, N], BF16, tag=f"qT{m}", name=f"qT{m}") for m in range(M)]
            kT_sb = [consts.tile([P, DT, N], BF16, tag=f"kT{m}", name=f"kT{m}") for m in range(M)]
            # v with per-head ones column (col 64) for the softmax denominator
            v_sb = [consts.tile([P, NT, H, HD + 1], BF16, tag=f"v{m}", name=f"v{m}") for m in range(M)]
            fused_sb = consts.tile([P, NT, C], F32, tag="fused", name="fused")
            fusedT_sb = consts.tile([P, CT, N], BF16, tag="fusedT", name="fusedT")
            out_sb = consts.tile([P, NT, C], F32, tag="outsb", name="outsb")
            ones_sb = consts.tile([1, P], BF16, tag="ones", name="ones")
            ident_sb = consts.tile([P, P], F32, tag="ident", name="ident")

            # ---- input DMAs (in first-use order) ----
            for m in range(M):
                nc.sync.dma_start(out=xt_sb[m], in_=xt_d[m])
                nc.sync.dma_start(out=wq_sb[m], in_=wq_d[m])
                nc.sync.dma_start(out=bq_sb[m], in_=bq_d[m])
                nc.sync.dma_start(out=wk_sb[m], in_=wk_d[m])
                nc.sync.dma_start(out=bk_sb[m], in_=bk_d[m])
                nc.sync.dma_start(out=wv_sb[m], in_=wv_d[m])
                nc.sync.dma_start(out=bv_sb[m], in_=bv_d[m])
            nc.sync.dma_start(out=wp_sb, in_=wp_d[:])
            nc.sync.dma_start(out=bp_sb, in_=bp_d[:])

            nc.gpsimd.memset(ones_sb, 1.0)
            make_identity(nc, ident_sb)

            def emit_proj(m):
                # q and k projections -> transposed layout [d, n]
                for w_sb, b_sb, dst in (
                    (wq_sb[m], bq_sb[m], qT_sb[m]),
                    (wk_sb[m], bk_sb[m], kT_sb[m]),
                ):
                    for dt in range(DT):
                        ps = ppsum.tile([P, N], F32, tag="proj", name="proj")
                        for ct in range(CT):
                            nc.tensor.matmul(
                                ps,
                                w_sb[:, ct, dt * P : (dt + 1) * P],
                                xt_sb[m][:, ct, :],
                                start=(ct == 0),
                                stop=(ct == CT - 1),
                            )
                        nc.vector.tensor_tensor(
                            dst[:, dt, :],
                            ps,
                            b_sb[:, dt : dt + 1].to_broadcast((P, N)),
                            AluOp.add,
                        )
                # v projection -> natural layout [n_k, d], bias via rank-1 matmul
                for nt in range(NT):
                    ps = ppsum.tile([P, C], F32, tag="proj", name="proj")
                    for ct in range(CT):
                        nc.tensor.matmul(
                            ps,
                            xt_sb[m][:, ct, nt * P : (nt + 1) * P],
                            wv_sb[m][:, ct, :],
                            start=(ct == 0),
                            stop=False,
                        )
                    nc.tensor.matmul(
                        ps, ones_sb[0:1, :], bv_sb[m][0:1, :], start=False, stop=True
                    )
                    nc.vector.tensor_copy(
                        out=v_sb[m][:, nt, :, 0:HD],
                        in_=ps.rearrange("p (h e) -> p h e", e=HD),
                    )
                # the per-head "ones" column
                nc.gpsimd.memset(v_sb[m][:, :, :, HD : HD + 1], ones_val)

            def emit_pair(i, j, first):
                w_ij = float(mw[i, j])
                for h in range(H):
                    hof = (h % 2) * HD
                    ht = h // 2
                    # 1-bank PV psum tile per head: [ns, 0:64]=PV, [ns, 64]=Z
                    pv_t = pvpsum.tile([P, NT, HD + 1], F32, tag="pv", name="pv")
                    e_t = esb.tile([P, CT, N], BF16, tag="E", name="E")
                    for half in range(2):
                        st = spsum.tile([P, 2, N], F32, tag="s", name="s")
                        for k2 in range(2):
                            kt = half * 2 + k2
                            nc.tensor.matmul(
                                st[:, k2, :],
                                kT_sb[j][hof : hof + HD, ht, kt * P : (kt + 1) * P],
                                qT_sb[i][hof : hof + HD, ht, :],
                                start=True,
                                stop=True,
                            )
                        nc.scalar.activation(
                            e_t[:, half * 2 : half * 2 + 2, :], st, ActFn.Exp
                        )
                    for ns in range(NT):
                        for kt in range(CT):
                            nc.tensor.matmul(
                                pv_t[:, ns, :],
                                e_t[:, kt, ns * P : (ns + 1) * P],
                                v_sb[j][:, kt, h, :],
                                start=(kt == 0),
                                stop=(kt == CT - 1),
                            )
                    # consume: fused[:, :, h] += pv[:, :, 0:64] / (Z * ones_val)
                    zr = zrp.tile([P, NT, 1], F32, tag="zr", name="zr")
                    nc.vector.reciprocal(zr, pv_t[:, :, HD : HD + 1])
                    if not uniform:
                        nc.vector.tensor_scalar(
                            zr, zr, w_ij / float(M), None, AluOp.mult
                        )
                    dst = fused_sb[:, :, h * HD : (h + 1) * HD]
                    if first:
                        nc.vector.tensor_tensor(
                            dst,
                            pv_t[:, :, 0:HD],
                            zr.to_broadcast((P, NT, HD)),
                            AluOp.mult,
                        )
                    else:
                        tmp = zrp.tile([P, NT, HD], F32, tag="ctmp", name="ctmp")
                        nc.vector.tensor_tensor(
                            tmp,
                            pv_t[:, :, 0:HD],
                            zr.to_broadcast((P, NT, HD)),
                            AluOp.mult,
                        )
                        nc.vector.tensor_tensor(dst, dst, tmp, AluOp.add)

            # ---- emission schedule (proj m interleaved with ready pairs) ----
            for _rep in range(reps):
                emit_proj(0)
                emit_proj(1)
                emit_pair(0, 0, True)
                emit_proj(2)
                for i, j in (
                    (0, 1), (1, 0), (1, 1), (0, 2), (2, 0), (1, 2), (2, 1), (2, 2)
                ):
                    emit_pair(i, j, False)

                # ---- transpose fused -> fusedT (bf16) via PE ----
                for ct in range(CT):
                    for nt in range(NT):
                        tpf = ppsum.tile([P, C], F32, tag="proj", name="proj")
                        tp = tpf[:, 0:P]
                        nc.tensor.transpose(
                            tp, fused_sb[:, nt, ct * P : (ct + 1) * P], ident_sb
                        )
                        nc.vector.tensor_copy(
                            out=fusedT_sb[:, ct, nt * P : (nt + 1) * P], in_=tp
                        )

                # ---- final projection ----
                for nt in range(NT):
                    ps = ppsum.tile([P, C], F32, tag="proj", name="proj")
                    for ct in range(CT):
                        nc.tensor.matmul(
                            ps,
                            fusedT_sb[:, ct, nt * P : (nt + 1) * P],
                            wp_sb[:, ct, :],
                            start=(ct == 0),
                            stop=False,
                        )
                    nc.tensor.matmul(
                        ps, ones_sb[0:1, :], bp_sb[0:1, :], start=False, stop=True
                    )
                    nc.vector.tensor_copy(out=out_sb[:, nt, :], in_=ps)

                nc.sync.dma_start(
                    out=out_d[:].rearrange("(nt p) c -> p nt c", p=P), in_=out_sb
                )

    nc.compile()
    return nc


def _prep_inputs(x, Wq, bq, Wk, bk, Wv, bv, mw, Wp, bp):
    """Host-side shard + retile. Returns (in_maps, uniform, ones_val)."""
    x = np.asarray(x, dtype=np.float32)
    Wq = np.asarray(Wq, dtype=np.float32)
    bq = np.asarray(bq, dtype=np.float32)
    Wk = np.asarray(Wk, dtype=np.float32)
    bk = np.asarray(bk, dtype=np.float32)
    Wv = np.asarray(Wv, dtype=np.float32)
    bv = np.asarray(bv, dtype=np.float32)
    mw = np.asarray(mw, dtype=np.float64)
    Wp = np.asarray(Wp, dtype=np.float32)
    bp = np.asarray(bp, dtype=np.float32)

    w0 = float(mw.flat[0])
    uniform = bool(np.all(mw == w0)) and abs(w0) > 1e-6
    if uniform:
        ones_val = float(NP_BF16(M / w0))
        # compensate bf16 rounding of ones_val exactly through Wp
        kappa = ones_val * w0 / M
    else:
        ones_val = 1.0
        kappa = 1.0

    def tile_w(w):  # [C, C] -> [P, CT, C]
        return np.ascontiguousarray(
            w.reshape(CT, P, C).transpose(1, 0, 2).astype(NP_BF16)
        )

    wq_h = np.stack([tile_w(Wq[m] * SCALE) for m in range(M)])
    wk_h = np.stack([tile_w(Wk[m]) for m in range(M)])
    wv_h = np.stack([tile_w(Wv[m]) for m in range(M)])
    wp_h = tile_w(Wp / kappa)

    def tile_b(b):  # [C] -> [P, DT]
        return np.ascontiguousarray(b.reshape(DT, P).T.astype(np.float32))

    bq_h = np.stack([tile_b(bq[m] * SCALE) for m in range(M)])
    bk_h = np.stack([tile_b(bk[m]) for m in range(M)])
    bv_h = np.ascontiguousarray(bv.reshape(M, 1, C).astype(NP_BF16))
    bp_h = np.ascontiguousarray(bp.reshape(1, C).astype(NP_BF16))

    # x [M,B,N,C] -> per-core xT [M,P,CT,N]
    xt_all = np.ascontiguousarray(
        x.transpose(1, 0, 3, 2)  # [B, M, C, N]
        .reshape(B, M, CT, P, N)
        .transpose(0, 1, 3, 2, 4)  # [B, M, P, CT, N]
        .astype(NP_BF16)
    )

    common = {
        "wq": wq_h,
        "wk": wk_h,
        "wv": wv_h,
        "wp": wp_h,
        "bq": bq_h,
        "bk": bk_h,
        "bv": bv_h,
        "bp": bp_h,
    }
    in_maps = [dict(common, xt=np.ascontiguousarray(xt_all[b])) for b in range(B)]
    return in_maps, uniform, ones_val, mw


def run(trace=False, **inputs):
    from concourse.bass_utils import run_bass_kernel_spmd

    in_maps, uniform, ones_val, mw = _prep_inputs(**inputs)
    nc = _build_bass(mw, uniform, ones_val)
    res = run_bass_kernel_spmd(
        nc, in_maps, core_ids=list(range(B)), trace=trace
    )
    out = np.stack([res.results[b]["out"] for b in range(B)]).astype(np.float32)
    return out, res


def kernel(**inputs):
    out, _ = run(trace=False, **inputs)
    return out


if __name__ == "__main__":
    rng = np.random.default_rng(0)
    ins = {
        "x": rng.standard_normal((M, B, N, C), dtype=np.float32),
        "Wq": rng.standard_normal((M, C, C), dtype=np.float32) * 0.02,
        "bq": rng.standard_normal((M, C), dtype=np.float32) * 0.02,
        "Wk": rng.standard_normal((M, C, C), dtype=np.float32) * 0.02,
        "bk": rng.standard_normal((M, C), dtype=np.float32) * 0.02,
        "Wv": rng.standard_normal((M, C, C), dtype=np.float32) * 0.02,
        "bv": rng.standard_normal((M, C), dtype=np.float32) * 0.02,
        "mw": np.ones((M, M), dtype=np.float32),
        "Wp": rng.standard_normal((C, C), dtype=np.float32) * 0.02,
        "bp": rng.standard_normal((C,), dtype=np.float32) * 0.02,
    }
    out = kernel(**ins)
    print("out", out.shape, out.dtype, float(np.abs(out).mean()))


# revision 12
# speedup vs baseline: 2.3844x; 2.3844x over previous
"""CrossModalAttention Trainium2 kernel.

Data-parallel over batch B=8 across the 8 NeuronCores (core b owns batch b,
weights replicated, no collectives). Within a core all 9 modality-pair
attentions run with bf16 matmuls / fp32 PSUM accumulation.

Layout strategy (per core, batch b fixed):
  xT[m]  : [c, n]  (c on partitions)  -- host pre-transposed, bf16
  qT/kT[m]: [d, n] = Wq[m].T-projection output, d on partitions
  v[m]   : [n_k, d] natural layout, with an extra per-head "ones" column so
           the PV matmul produces the softmax denominator Z in column 64.
  S^T    : [k, n] per (i,j,head) from lhsT=kT-slice, rhs=qT-slice (K=64)
  E = exp(S^T) via ACT, bf16
  PV     : out[n-sub, 65] with lhsT=E-slice, rhs=v-slice(65 cols) accumulated
           over key tiles; col 64 = Z * ones_val.
  consume: fused[n, c] += PV[:, 0:64] * reciprocal(Z*ones_val)  (DVE)
  final  : PE-transpose fused -> fusedT, out = fusedT.T @ Wp + bp
"""

import sys

import numpy as np

for _p in ("/opt/trn_rl_repo",):
    if _p not in sys.path:
        sys.path.insert(0, _p)

import ml_dtypes  # noqa: E402

import concourse.bass as bass  # noqa: E402
from concourse import bacc  # noqa: E402
import concourse.mybir as mybir  # noqa: E402
import concourse.tile as tile  # noqa: E402

M, B, N, C, H = 3, 8, 512, 512, 8
HD = C // H  # 64
P = 128
CT = C // P  # 4 contraction tiles
NT = N // P  # 4 row tiles
DT = C // P  # 4 output-channel tiles
SCALE = float(HD) ** -0.5

BF16 = mybir.dt.bfloat16
F32 = mybir.dt.float32
NP_BF16 = ml_dtypes.bfloat16

AluOp = mybir.AluOpType
ActFn = mybir.ActivationFunctionType


def _build_bass(mw, uniform, ones_val, reps=1, skip=()):
    """Emit the single-core SPMD program. mw is the [M,M] modal weight matrix
    (values are baked into the program as immediates)."""
    from concourse.masks import make_identity

    nc = bacc.Bacc(None)

    xt_d = nc.dram_tensor("xt", [M, P, CT, N], BF16, kind="ExternalInput")
    wq_d = nc.dram_tensor("wq", [M, P, CT, C], BF16, kind="ExternalInput")
    wk_d = nc.dram_tensor("wk", [M, P, CT, C], BF16, kind="ExternalInput")
    wv_d = nc.dram_tensor("wv", [M, P, CT, C], BF16, kind="ExternalInput")
    wp_d = nc.dram_tensor("wp", [P, CT, C], BF16, kind="ExternalInput")
    bq_d = nc.dram_tensor("bq", [M, P, DT], F32, kind="ExternalInput")
    bk_d = nc.dram_tensor("bk", [M, P, DT], F32, kind="ExternalInput")
    bv_d = nc.dram_tensor("bv", [M, 1, C], BF16, kind="ExternalInput")
    bp_d = nc.dram_tensor("bp", [1, C], BF16, kind="ExternalInput")
    out_d = nc.dram_tensor("out", [N, C], F32, kind="ExternalOutput")

    with tile.TileContext(nc) as tc:
        with (
            tc.tile_pool(name="consts", bufs=1) as consts,
            tc.tile_pool(name="esb", bufs=3) as esb,
            tc.tile_pool(name="zr", bufs=8) as zrp,
            tc.tile_pool(name="ppsum", bufs=2, space="PSUM") as ppsum,
            tc.tile_pool(name="spsum", bufs=2, space="PSUM") as spsum,
            tc.tile_pool(name="pvpsum", bufs=2, space="PSUM") as pvpsum,
        ):
            # ---- persistent SBUF tiles ----
            xt_sb = [consts.tile([P, CT, N], BF16, tag=f"xt{m}", name=f"xt{m}") for m in range(M)]
            wq_sb = [consts.tile([P, CT, C], BF16, tag=f"wq{m}", name=f"wq{m}") for m in range(M)]
            wk_sb = [consts.tile([P, CT, C], BF16, tag=f"wk{m}", name=f"wk{m}") for m in range(M)]
            wv_sb = [consts.tile([P, CT, C], BF16, tag=f"wv{m}", name=f"wv{m}") for m in range(M)]
            wp_sb = consts.tile([P, CT, C], BF16, tag="wp", name="wp")
            bq_sb = [consts.tile([P, DT], F32, tag=f"bq{m}", name=f"bq{m}") for m in range(M)]
            bk_sb = [consts.tile([P, DT], F32, tag=f"bk{m}", name=f"bk{m}") for m in range(M)]
            bv_sb = [consts.tile([1, C], BF16, tag=f"bv{m}", name=f"bv{m}") for m in range(M)]
            bp_sb = consts.tile([1, C], BF16, tag="bp", name="bp")
            qT_sb = [consts.tile([P, DT, N], BF16, tag=f"qT{m}", name=f"qT{m}") for m in range(M)]
            kT_sb = [consts.tile([P, DT, N], BF16, tag=f"kT{m}", name=f"kT{m}") for m in range(M)]
            # v with per-head ones column (col 64) for the softmax denominator
            v_sb = [consts.tile([P, NT, H, HD + 1], BF16, tag=f"v{m}", name=f"v{m}") for m in range(M)]
            fused_sb = consts.tile([P, NT, C], F32, tag="fused", name="fused")
            fusedT_sb = consts.tile([P, CT, N], BF16, tag="fusedT", name="fusedT")
            out_sb = consts.tile([P, NT, C], F32, tag="outsb", name="outsb")
            ones_sb = consts.tile([1, P], BF16, tag="ones", name="ones")
            ident_sb = consts.tile([P, P], F32, tag="ident", name="ident")

            # ---- input DMAs (in first-use order) ----
            for m in range(M):
                nc.sync.dma_start(out=xt_sb[m], in_=xt_d[m])
                nc.sync.dma_start(out=wq_sb[m], in_=wq_d[m])
                nc.sync.dma_start(out=bq_sb[m], in_=bq_d[m])
                nc.sync.dma_start(out=wk_sb[m], in_=wk_d[m])
                nc.sync.dma_start(out=bk_sb[m], in_=bk_d[m])
                nc.sync.dma_start(out=wv_sb[m], in_=wv_d[m])
                nc.sync.dma_start(out=bv_sb[m], in_=bv_d[m])
            nc.sync.dma_start(out=wp_sb, in_=wp_d[:])
            nc.sync.dma_start(out=bp_sb, in_=bp_d[:])

            nc.gpsimd.memset(ones_sb, 1.0)
            make_identity(nc, ident_sb)

            def emit_proj(m):
                # q and k projections -> transposed layout [d, n]
                for w_sb, b_sb, dst in (
                    (wq_sb[m], bq_sb[m], qT_sb[m]),
                    (wk_sb[m], bk_sb[m], kT_sb[m]),
                ):
                    for dt in range(DT):
                        ps = ppsum.tile([P, N], F32, tag="proj", name="proj")
                        for ct in range(CT):
                            nc.tensor.matmul(
                                ps,
                                w_sb[:, ct, dt * P : (dt + 1) * P],
                                xt_sb[m][:, ct, :],
                                start=(ct == 0),
                                stop=(ct == CT - 1),
                            )
                        nc.vector.tensor_tensor(
                            dst[:, dt, :],
                            ps,
                            b_sb[:, dt : dt + 1].to_broadcast((P, N)),
                            AluOp.add,
                        )
                # v projection -> natural layout [n_k, d], bias via rank-1 matmul
                for nt in range(NT):
                    ps = ppsum.tile([P, C], F32, tag="proj", name="proj")
                    for ct in range(CT):
                        nc.tensor.matmul(
                            ps,
                            xt_sb[m][:, ct, nt * P : (nt + 1) * P],
                            wv_sb[m][:, ct, :],
                            start=(ct == 0),
                            stop=False,
                        )
                    nc.tensor.matmul(
                        ps, ones_sb[0:1, :], bv_sb[m][0:1, :], start=False, stop=True
                    )
                    nc.vector.tensor_copy(
                        out=v_sb[m][:, nt, :, 0:HD],
                        in_=ps.rearrange("p (h e) -> p h e", e=HD),
                    )
                # the per-head "ones" column
                nc.gpsimd.memset(v_sb[m][:, :, :, HD : HD + 1], ones_val)

            def emit_qk_exp(i, j, h):
                hof = (h % 2) * HD
                ht = h // 2
                e_t = esb.tile([P, CT, N], BF16, tag="E", name="E")
                if "qk" in skip:
                    nc.gpsimd.memset(e_t, 1.0)
                for half in range(2):
                    if "qk" in skip:
                        continue
                    st = spsum.tile([P, 2, N], F32, tag="s", name="s")
                    for k2 in range(2):
                        kt = half * 2 + k2
                        nc.tensor.matmul(
                            st[:, k2, :],
                            kT_sb[j][hof : hof + HD, ht, kt * P : (kt + 1) * P],
                            qT_sb[i][hof : hof + HD, ht, :],
                            start=True,
                            stop=True,
                        )
                    if "exp" not in skip:
                        nc.scalar.activation(
                            e_t[:, half * 2 : half * 2 + 2, :], st, ActFn.Exp
                        )
                return e_t

            def emit_pv_consume(i, j, h, e_t, first):
                w_ij = float(mw[i, j])
                pv_t = pvpsum.tile([P, NT, HD + 1], F32, tag="pv", name="pv")
                if "pv" not in skip:
                    for ns in range(NT):
                        for kt in range(CT):
                            nc.tensor.matmul(
                                pv_t[:, ns, :],
                                e_t[:, kt, ns * P : (ns + 1) * P],
                                v_sb[j][:, kt, h, :],
                                start=(kt == 0),
                                stop=(kt == CT - 1),
                            )
                # consume: fused[:, :, h] += pv[:, :, 0:64] / (Z * ones_val)
                if "consume" in skip:
                    if first and h == 0:
                        nc.gpsimd.memset(fused_sb, 0.5)
                    return
                zr = zrp.tile([P, NT, 1], F32, tag="zr", name="zr")
                nc.vector.reciprocal(zr, pv_t[:, :, HD : HD + 1])
                if not uniform:
                    nc.vector.tensor_scalar(
                        zr, zr, w_ij / float(M), None, AluOp.mult
                    )
                dst = fused_sb[:, :, h * HD : (h + 1) * HD]
                if first:
                    nc.vector.tensor_tensor(
                        dst,
                        pv_t[:, :, 0:HD],
                        zr.to_broadcast((P, NT, HD)),
                        AluOp.mult,
                    )
                else:
                    tmp = zrp.tile([P, NT, HD], F32, tag="ctmp", name="ctmp")
                    nc.vector.tensor_tensor(
                        tmp,
                        pv_t[:, :, 0:HD],
                        zr.to_broadcast((P, NT, HD)),
                        AluOp.mult,
                    )
                    nc.vector.tensor_tensor(dst, dst, tmp, AluOp.add)

            # ---- emission schedule: 1-deep software pipeline over tasks ----
            pair_order = (
                (0, 0), (0, 1), (1, 0), (1, 1),
                (0, 2), (2, 0), (1, 2), (2, 1), (2, 2),
            )
            tasks = [(i, j, h) for (i, j) in pair_order for h in range(H)]
            for _rep in range(reps):
                emit_proj(0)
                emit_proj(1)
                prev = None
                for idx, (i, j, h) in enumerate(tasks):
                    if idx == H:
                        emit_proj(2)
                    e_t = emit_qk_exp(i, j, h)
                    if prev is not None:
                        pi, pj, ph, pe = prev
                        emit_pv_consume(pi, pj, ph, pe, first=(pi == 0 and pj == 0))
                    prev = (i, j, h, e_t)
                pi, pj, ph, pe = prev
                emit_pv_consume(pi, pj, ph, pe, first=False)

                # ---- transpose fused -> fusedT (bf16) via PE ----
                for ct in range(CT):
                    for nt in range(NT):
                        tpf = ppsum.tile([P, C], F32, tag="proj", name="proj")
                        tp = tpf[:, 0:P]
                        nc.tensor.transpose(
                            tp, fused_sb[:, nt, ct * P : (ct + 1) * P], ident_sb
                        )
                        nc.vector.tensor_copy(
                            out=fusedT_sb[:, ct, nt * P : (nt + 1) * P], in_=tp
                        )

                # ---- final projection ----
                for nt in range(NT):
                    ps = ppsum.tile([P, C], F32, tag="proj", name="proj")
                    for ct in range(CT):
                        nc.tensor.matmul(
                            ps,
                            fusedT_sb[:, ct, nt * P : (nt + 1) * P],
                            wp_sb[:, ct, :],
                            start=(ct == 0),
                            stop=False,
                        )
                    nc.tensor.matmul(
                        ps, ones_sb[0:1, :], bp_sb[0:1, :], start=False, stop=True
                    )
                    nc.vector.tensor_copy(out=out_sb[:, nt, :], in_=ps)

                nc.sync.dma_start(
                    out=out_d[:].rearrange("(nt p) c -> p nt c", p=P), in_=out_sb
                )

    nc.compile()
    return nc


def _prep_inputs(x, Wq, bq, Wk, bk, Wv, bv, mw, Wp, bp):
    """Host-side shard + retile. Returns (in_maps, uniform, ones_val)."""
    x = np.asarray(x, dtype=np.float32)
    Wq = np.asarray(Wq, dtype=np.float32)
    bq = np.asarray(bq, dtype=np.float32)
    Wk = np.asarray(Wk, dtype=np.float32)
    bk = np.asarray(bk, dtype=np.float32)
    Wv = np.asarray(Wv, dtype=np.float32)
    bv = np.asarray(bv, dtype=np.float32)
    mw = np.asarray(mw, dtype=np.float64)
    Wp = np.asarray(Wp, dtype=np.float32)
    bp = np.asarray(bp, dtype=np.float32)

    w0 = float(mw.flat[0])
    uniform = bool(np.all(mw == w0)) and abs(w0) > 1e-6
    if uniform:
        ones_val = float(NP_BF16(M / w0))
        # compensate bf16 rounding of ones_val exactly through Wp
        kappa = ones_val * w0 / M
    else:
        ones_val = 1.0
        kappa = 1.0

    def tile_w(w):  # [C, C] -> [P, CT, C]
        return np.ascontiguousarray(
            w.reshape(CT, P, C).transpose(1, 0, 2).astype(NP_BF16)
        )

    wq_h = np.stack([tile_w(Wq[m] * SCALE) for m in range(M)])
    wk_h = np.stack([tile_w(Wk[m]) for m in range(M)])
    wv_h = np.stack([tile_w(Wv[m]) for m in range(M)])
    wp_h = tile_w(Wp / kappa)

    def tile_b(b):  # [C] -> [P, DT]
        return np.ascontiguousarray(b.reshape(DT, P).T.astype(np.float32))

    bq_h = np.stack([tile_b(bq[m] * SCALE) for m in range(M)])
    bk_h = np.stack([tile_b(bk[m]) for m in range(M)])
    bv_h = np.ascontiguousarray(bv.reshape(M, 1, C).astype(NP_BF16))
    bp_h = np.ascontiguousarray(bp.reshape(1, C).astype(NP_BF16))

    # x [M,B,N,C] -> per-core xT [M,P,CT,N]
    xt_all = np.ascontiguousarray(
        x.transpose(1, 0, 3, 2)  # [B, M, C, N]
        .reshape(B, M, CT, P, N)
        .transpose(0, 1, 3, 2, 4)  # [B, M, P, CT, N]
        .astype(NP_BF16)
    )

    common = {
        "wq": wq_h,
        "wk": wk_h,
        "wv": wv_h,
        "wp": wp_h,
        "bq": bq_h,
        "bk": bk_h,
        "bv": bv_h,
        "bp": bp_h,
    }
    in_maps = [dict(common, xt=np.ascontiguousarray(xt_all[b])) for b in range(B)]
    return in_maps, uniform, ones_val, mw


def run(trace=False, **inputs):
    from concourse.bass_utils import run_bass_kernel_spmd

    in_maps, uniform, ones_val, mw = _prep_inputs(**inputs)
    nc = _build_bass(mw, uniform, ones_val)
    res = run_bass_kernel_spmd(
        nc, in_maps, core_ids=list(range(B)), trace=trace
    )
    out = np.stack([res.results[b]["out"] for b in range(B)]).astype(np.float32)
    return out, res


def kernel(**inputs):
    out, _ = run(trace=False, **inputs)
    return out


if __name__ == "__main__":
    rng = np.random.default_rng(0)
    ins = {
        "x": rng.standard_normal((M, B, N, C), dtype=np.float32),
        "Wq": rng.standard_normal((M, C, C), dtype=np.float32) * 0.02,
        "bq": rng.standard_normal((M, C), dtype=np.float32) * 0.02,
        "Wk": rng.standard_normal((M, C, C), dtype=np.float32) * 0.02,
        "bk": rng.standard_normal((M, C), dtype=np.float32) * 0.02,
        "Wv": rng.standard_normal((M, C, C), dtype=np.float32) * 0.02,
        "bv": rng.standard_normal((M, C), dtype=np.float32) * 0.02,
        "mw": np.ones((M, M), dtype=np.float32),
        "Wp": rng.standard_normal((C, C), dtype=np.float32) * 0.02,
        "bp": rng.standard_normal((C,), dtype=np.float32) * 0.02,
    }
    out = kernel(**ins)
    print("out", out.shape, out.dtype, float(np.abs(out).mean()))
